# revision 1
# baseline (speedup 1.0000x reference)
"""Trainium2 Bass kernel for nn_MultiHeadedAttention_6416681140387.

Two-branch windowed video attention:
  x [8,256,96,96] -> 1x1 conv Q/K/V -> per-branch full attention over
  window-token features (branch0: 4x4 patches, d=2048, 2304 key tokens;
  branch1: 8x8 patches, d=8192, 576 key tokens) -> concat channels
  -> 3x3 conv + LeakyReLU(0.2).

Sharding: 8 cores = (video b in {0,1}) x (frame t in {0..3}). Each core
computes its full output frame [256,96,96]; K/V are recomputed per core from
its 4-frame video slice (no collectives). Host stacks the 8 frames.

Numerics: conv matmuls run in float32r (full-rate 4-byte PE mode); the
attention path (Q/K scores, P, V) runs in bf16 with fp32 PSUM accumulation.
Branch1 key tokens are padded 144->160 per frame so frame boundaries stay
32-aligned in the 128-partition PV tiling; padded scores are -1e30 -> P=0.
"""

import sys

if "/opt/trn_rl_repo" not in sys.path:
    sys.path.insert(0, "/opt/trn_rl_repo")

import math
from contextlib import ExitStack

import numpy as np

import concourse.bass as bass
import concourse.tile as tile
from concourse import bacc, mybir
from concourse.masks import make_identity

F32 = mybir.dt.float32
F32R = mybir.dt.float32r
BF16 = mybir.dt.bfloat16

T = 4
C = 256
H = W = 96
PIX = H * W
NCORES = 8

PSZ = [4, 8]
OHB = [24, 12]                  # token grid side per branch
NTF = [576, 144]                # real tokens per frame
NTFP = [576, 160]               # padded tokens per frame
NKP = [2304, 640]               # padded key tokens per video
NQ = [576, 144]                 # query tokens (one frame)
NCH = [16, 64]                  # d-chunks (psz^2)
SC = [1.0 / math.sqrt(2048.0), 1.0 / math.sqrt(8192.0)]
NQB = [[(0, 128), (128, 128), (256, 128), (384, 128), (512, 64)],
       [(0, 128), (128, 16)]]
NEG = -1.0e30

Exp = mybir.ActivationFunctionType.Exp
Identity = mybir.ActivationFunctionType.Identity


def _subpieces(br):
    """V/PT chunk tiles: list over tiles ti of list of sub-pieces
    (kf, ftok0, m, off). Partition offsets obey the PE col-group rule:
    off 0 -> m<=128, off 64 -> m<=64, off 32/96 -> m<=32."""
    ntiles = NKP[br] // 128
    out = []
    for ti in range(ntiles):
        lo, hi = ti * 128, ti * 128 + 128
        pieces = []
        for kf in range(T):
            f0 = kf * NTFP[br]
            a, b = max(lo, f0), min(hi, f0 + NTF[br])
            while a < b:
                off = a - lo
                cap = 128 - off if off == 0 else (64 if off == 64 else 32)
                m = min(b - a, cap)
                pieces.append((kf, a - f0, m, off))
                a += m
        out.append(pieces)
    return out


def _pad_rows(br, ti, pieces):
    """Partition ranges of V tile ti not covered by real tokens."""
    used = sorted((off, off + m) for (_, _, m, off) in pieces)
    gaps, pos = [], 0
    for a, b in used:
        if a > pos:
            gaps.append((pos, a))
        pos = b
    if pos < 128:
        gaps.append((pos, 128))
    return gaps


PHASES = {"A", "SM", "C0", "C1", "D"}


def build(nc):
    xv = nc.dram_tensor("xv", [T, C, PIX], F32R, kind="ExternalInput")
    xf = nc.dram_tensor("xf", [C, PIX], F32R, kind="ExternalInput")
    wqt = nc.dram_tensor("wqt", [C, C], F32R, kind="ExternalInput")
    wkt = nc.dram_tensor("wkt", [C, C], F32R, kind="ExternalInput")
    wvt = nc.dram_tensor("wvt", [C, C], F32R, kind="ExternalInput")
    wot = nc.dram_tensor("wot", [9, C, C], F32R, kind="ExternalInput")
    bq = nc.dram_tensor("bq", [C], F32, kind="ExternalInput")
    bk = nc.dram_tensor("bk", [C], F32, kind="ExternalInput")
    bv = nc.dram_tensor("bv", [C], F32, kind="ExternalInput")
    bo = nc.dram_tensor("bo", [C], F32, kind="ExternalInput")
    out = nc.dram_tensor("out", [C, PIX], F32, kind="ExternalOutput")

    alt = [0]

    def bias_copy_alt(dst, src, bias_ap):
        alt[0] ^= 1
        if alt[0]:
            nc.scalar.activation(out=dst, in_=src, func=Identity,
                                 bias=bias_ap, scale=1.0)
        else:
            nc.vector.tensor_scalar_add(dst, src, bias_ap)

    rr = [0]

    def copy_rr(dst, src):
        rr[0] = (rr[0] + 1) % 3
        if rr[0] == 0:
            nc.vector.tensor_copy(dst, src)
        elif rr[0] == 1:
            nc.scalar.copy(dst, src)
        else:
            nc.gpsimd.tensor_copy(dst, src)

    with tile.TileContext(nc, pool_alloc_mode="queue") as tc, ExitStack() as top:
        persist = top.enter_context(tc.tile_pool(name="persist", bufs=1))
        dramp = top.enter_context(tc.tile_pool(name="dram", bufs=1, space="DRAM"))

        wq_sb, wk_sb, wv_sb = [None, None], [None, None], [None, None]
        for name, dt_, lst in (("wq", wqt, wq_sb), ("wk", wkt, wk_sb),
                               ("wv", wvt, wv_sb)):
            for cb in range(2):
                t = persist.tile([128, C], F32R, name=f"{name}{cb}", tag=f"{name}{cb}")
                nc.sync.dma_start(out=t, in_=dt_.ap()[cb * 128:(cb + 1) * 128, :])
                lst[cb] = t
        wv_bf = []
        for cb in range(2):
            t = persist.tile([128, C], BF16, name=f"wvbf{cb}", tag=f"wvbf{cb}")
            nc.vector.tensor_copy(t, wv_sb[cb])
            wv_bf.append(t)

        def bias_tile(name, dt_):
            t = persist.tile([128, 2], F32, tag=name)
            nc.sync.dma_start(
                out=t, in_=bass.AP(tensor=dt_.ap().tensor, offset=0,
                                   ap=[[1, 128], [128, 2]]))
            return t

        bq_sb = bias_tile("bq", bq)
        bk_sb = bias_tile("bk", bk)
        bo_sb = bias_tile("bo", bo)
        bv_sb = bias_tile("bv", bv)
        ident = persist.tile([128, 128], BF16, name="ident", tag="ident")
        make_identity(nc, ident)
        zrow = persist.tile([128, 98], F32, name="zrow", tag="zrow")
        nc.vector.memset(zrow, 0.0)

        def conv1x1(x2d, w_sb, b_sb, out_tiles, xs_pool, ps_pool):
            """x2d [256, 9216] fp32 -> out_tiles bf16 [2][128, 9216], + bias."""
            for ch in range(6):
                xt = []
                for cb in range(2):
                    t = xs_pool.tile([128, 1536], F32R, name=f"xs{cb}",
                                     tag=f"xs{cb}", bufs=2)
                    nc.sync.dma_start(
                        out=t, in_=x2d[cb * 128:(cb + 1) * 128,
                                       ch * 1536:(ch + 1) * 1536])
                    xt.append(t)
                for coutb in range(2):
                    for pb in range(3):
                        ps = ps_pool.tile([128, 512], F32, name="cps", tag="cps")
                        for cb in range(2):
                            nc.tensor.matmul(
                                ps, w_sb[cb][:, coutb * 128:(coutb + 1) * 128],
                                xt[cb][:, pb * 512:(pb + 1) * 512],
                                start=(cb == 0), stop=(cb == 1))
                        o = ch * 1536 + pb * 512
                        bias_copy_alt(out_tiles[coutb][:, o:o + 512], ps,
                                      b_sb[:, coutb:coutb + 1])

        # ---------------- phases Q + A: Q/K conv and scores ----------------
        # pool open order = reverse close order (LIFO):
        #   PT1 (lives to end) < PT0 (to end of PV0) < P (to end of
        #   transposes) < S (to end of softmax) < qw (to end of A)
        esPT1 = ExitStack()
        p_PT1 = esPT1.enter_context(tc.tile_pool(name="PT1", bufs=1))
        pt1_t = [p_PT1.tile([128, NQ[1]], BF16, name=f"pt1_{i}", tag=f"pt1_{i}")
                 for i in range(NKP[1] // 128)]
        esPT0 = ExitStack()
        p_PT0 = esPT0.enter_context(tc.tile_pool(name="PT0", bufs=1))
        pt0_t = [p_PT0.tile([128, NQ[0]], BF16, name=f"pt0_{i}", tag=f"pt0_{i}")
                 for i in range(NKP[0] // 128)]
        pt_t = [pt0_t, pt1_t]
        esP = ExitStack()
        p_P = esP.enter_context(tc.tile_pool(name="P", bufs=1))
        p_t = [[p_P.tile([128, NKP[b]], BF16, name=f"p{b}_{i}", tag=f"p{b}_{i}")
                for i in range(len(NQB[b]))] for b in range(2)]
        esQW = ExitStack()
        p_qw = esQW.enter_context(tc.tile_pool(name="qw", bufs=1))
        qw = [p_qw.tile([128, NCH[b] * NTF[b]], BF16, name=f"qw{b}", tag=f"qw{b}")
              for b in range(2)]
        p_run = esQW.enter_context(tc.tile_pool(name="run", bufs=1))
        run_mx = [[p_run.tile([128, 1], F32, name=f"mx{b}_{i}", tag=f"mx{b}_{i}")
                   for i in range(len(NQB[b]))] for b in range(2)]
        run_ls = [[p_run.tile([128, 1], F32, name=f"ls{b}_{i}", tag=f"ls{b}_{i}")
                   for i in range(len(NQB[b]))] for b in range(2)]
        # branch1 pad columns of P stay 0 through the online rescales
        for i in range(len(NQB[1])):
            for kf in range(T):
                nc.gpsimd.memset(
                    p_t[1][i][:, kf * 160 + 144:(kf + 1) * 160], 0.0)

        with tc.tile_pool(name="qcm", bufs=1) as p_qcm, \
             tc.tile_pool(name="qxs", bufs=1) as p_qxs, \
             tc.tile_pool(name="qps", bufs=2, space="PSUM") as p_qps:
            q_cm = [p_qcm.tile([128, PIX], BF16, name=f"qcm{cb}", tag=f"qcm{cb}")
                    for cb in range(2)]
            conv1x1(xf.ap(), wq_sb, bq_sb, q_cm, p_qxs, p_qps)
            for b in range(2):
                psz, ohb = PSZ[b], OHB[b]
                qv = q_cm[b].rearrange("p (oh hh ow ww) -> p oh hh ow ww",
                                       oh=ohb, hh=psz, ow=ohb, ww=psz)
                for ci in range(NCH[b]):
                    wy, wx = divmod(ci, psz)
                    dst = qw[b][:, ci * NTF[b]:(ci + 1) * NTF[b]].rearrange(
                        "p (a c) -> p a c", a=ohb)
                    copy_rr(dst, qv[:, :, wy, :, wx])

        p_stat = esQW.enter_context(tc.tile_pool(name="stat", bufs=4))
        with tc.tile_pool(name="kcm", bufs=1) as p_kcm, \
             tc.tile_pool(name="kxs", bufs=1) as p_kxs, \
             tc.tile_pool(name="kps", bufs=2, space="PSUM") as p_kps, \
             tc.tile_pool(name="sps0", bufs=3, space="PSUM") as p_sps0, \
             tc.tile_pool(name="sps1", bufs=2, space="PSUM") as p_sps1:
            for kf in range(T):
                k_cm = [p_kcm.tile([128, PIX], BF16, name=f"kcm{cb}",
                                   tag=f"kcm{cb}") for cb in range(2)]
                conv1x1(xv.ap()[kf], wk_sb, bk_sb, k_cm, p_kxs, p_kps)
                for b in range(2):
                    psz, ohb, ntf = PSZ[b], OHB[b], NTF[b]
                    kv = k_cm[b].rearrange(
                        "p (oh hh ow ww) -> p oh hh ow ww",
                        oh=ohb, hh=psz, ow=ohb, ww=psz)
                    nmk = 2 if b == 0 else 1
                    mkw = ntf // nmk              # 288 / 144
                    for nqi, (q0, nqsz) in enumerate(NQB[b]):
                        for mkh in range(nmk):
                            ps = (p_sps0 if b == 0 else p_sps1).tile(
                                [128, mkw], F32, name=f"sps{b}", tag=f"sps{b}")
                            oh0 = mkh * (ohb // nmk)
                            for ci in range(NCH[b]):
                                wy, wx = divmod(ci, psz)
                                rhs = kv[:, oh0:oh0 + ohb // nmk, wy, :, wx]
                                lhsT = qw[b][:, ci * ntf + q0:
                                             ci * ntf + q0 + nqsz]
                                nc.tensor.matmul(
                                    ps[:nqsz], lhsT, rhs,
                                    start=(ci == 0), stop=(ci == NCH[b] - 1))
                            # online softmax over key blocks
                            o = kf * NTFP[b] + mkh * mkw
                            pt = p_t[b][nqi]
                            mx, ls = run_mx[b][nqi], run_ls[b][nqi]
                            bm = p_stat.tile([128, 1], F32, name="bm",
                                             tag="bm")
                            nc.vector.reduce_max(out=bm[:nqsz],
                                                 in_=ps[:nqsz, :],
                                                 axis=mybir.AxisListType.X)
                            first = (kf == 0 and mkh == 0)
                            if first:
                                nc.vector.tensor_copy(mx[:nqsz], bm[:nqsz])
                                nmx = p_stat.tile([128, 1], F32, name="nmx",
                                                  tag="nmx")
                                nc.vector.tensor_scalar_mul(
                                    nmx[:nqsz], mx[:nqsz], -SC[b])
                                nc.scalar.activation(
                                    out=pt[:nqsz, o:o + mkw],
                                    in_=ps[:nqsz, :], func=Exp,
                                    bias=nmx[:nqsz], scale=SC[b],
                                    accum_out=ls[:nqsz])
                            else:
                                nmax = p_stat.tile([128, 1], F32,
                                                   name="nmax", tag="nmax")
                                nc.vector.tensor_max(nmax[:nqsz], mx[:nqsz],
                                                     bm[:nqsz])
                                nmx = p_stat.tile([128, 1], F32, name="nmx",
                                                  tag="nmx")
                                nc.vector.tensor_scalar_mul(
                                    nmx[:nqsz], nmax[:nqsz], -SC[b])
                                delta = p_stat.tile([128, 1], F32,
                                                    name="delta", tag="delta")
                                nc.scalar.activation(
                                    out=delta[:nqsz], in_=mx[:nqsz],
                                    func=Exp, bias=nmx[:nqsz], scale=SC[b])
                                # rescale previously written P columns
                                nc.vector.tensor_scalar_mul(
                                    pt[:nqsz, 0:o], pt[:nqsz, 0:o],
                                    delta[:nqsz])
                                pl = p_stat.tile([128, 1], F32, name="pl",
                                                 tag="pl")
                                nc.scalar.activation(
                                    out=pt[:nqsz, o:o + mkw],
                                    in_=ps[:nqsz, :], func=Exp,
                                    bias=nmx[:nqsz], scale=SC[b],
                                    accum_out=pl[:nqsz])
                                nc.vector.scalar_tensor_tensor(
                                    out=ls[:nqsz], in0=ls[:nqsz],
                                    scalar=delta[:nqsz], in1=pl[:nqsz],
                                    op0=mybir.AluOpType.mult,
                                    op1=mybir.AluOpType.add)
                                nc.vector.tensor_copy(mx[:nqsz], nmax[:nqsz])
        # final normalization of P
        if "SM" not in PHASES:
            esQW.close(); esP.close(); esPT0.close(); esPT1.close()
            return nc
        for b in range(2):
            for nqi, (q0, nqsz) in enumerate(NQB[b]):
                rs = p_stat.tile([128, 1], F32, name="rs", tag="rs")
                nc.vector.reciprocal(rs[:nqsz], run_ls[b][nqi][:nqsz])
                nc.vector.tensor_scalar_mul(
                    p_t[b][nqi][:nqsz, :], p_t[b][nqi][:nqsz, :], rs[:nqsz])
        esQW.close()

        # ---------------- P^T transposes for both branches ----------------
        with tc.tile_pool(name="ptps", bufs=2, space="PSUM") as p_ptps:
            for br in range(2):
                if f"C{br}" not in PHASES:
                    continue
                for ti in range(NKP[br] // 128):
                    for nqi, (q0, nqsz) in enumerate(NQB[br]):
                        tp = p_ptps.tile([128, 128], BF16, name="ptps",
                                         tag="ptps")
                        nc.tensor.transpose(
                            tp[:, :nqsz],
                            p_t[br][nqi][:nqsz, ti * 128:(ti + 1) * 128],
                            ident[:nqsz, :nqsz])
                        alt[0] ^= 1
                        if alt[0]:
                            nc.scalar.copy(pt_t[br][ti][:, q0:q0 + nqsz],
                                           tp[:, :nqsz])
                        else:
                            nc.vector.tensor_copy(
                                pt_t[br][ti][:, q0:q0 + nqsz], tp[:, :nqsz])
        esP.close()

        # ---------------- phase C: V build + PV, per branch ----------------
        att0_dram = dramp.tile([128, 98 * 98], F32R, name="att0d", tag="att0d")
        esAtt1 = ExitStack()
        att_sb = {}

        for br in range(2):
            if f"C{br}" not in PHASES:
                continue
            psz, ohb, ntf = PSZ[br], OHB[br], NTF[br]
            sub = _subpieces(br)
            ntiles = len(sub)
            if br == 1:
                # att1 outlives V1 (used directly by phase D) -> open first
                p_att1 = esAtt1.enter_context(tc.tile_pool(name="att1", bufs=1))
            esV = ExitStack()
            p_V = esV.enter_context(tc.tile_pool(name=f"V{br}", bufs=1))
            v_t = [p_V.tile([128, NCH[br] * 128], BF16, name=f"v{br}_{i}",
                            tag=f"v{br}_{i}") for i in range(ntiles)]
            for ti in range(ntiles):
                if _pad_rows(br, ti, sub[ti]):
                    nc.gpsimd.memset(v_t[ti][:, :], 0.0)

            # --- V conv: x gathered window-major (bf16), x stationary ---
            with tc.tile_pool(name=f"xw{br}", bufs=1) as p_xw, \
                 tc.tile_pool(name=f"xl{br}", bufs=1) as p_xl, \
                 tc.tile_pool(name=f"vps{br}", bufs=4, space="PSUM") as p_vps:
                for kf in range(T):
                    xw = [p_xw.tile([128, NCH[br] * NTF[br]], BF16,
                                    name=f"xw{cb}", tag=f"xw{cb}")
                          for cb in range(2)]
                    nql = 4
                    csz = PIX // nql
                    ohc = ohb // nql
                    tpc = NTF[br] // nql
                    for chq in range(nql):
                        for cb in range(2):
                            xt = p_xl.tile([128, csz], F32R, name=f"xl{cb}",
                                           tag=f"xl{cb}",
                                           bufs=2 if br == 0 else 1)
                            nc.sync.dma_start(
                                out=xt,
                                in_=xv.ap()[kf, cb * 128:(cb + 1) * 128,
                                            chq * csz:(chq + 1) * csz])
                            xtv = xt.rearrange(
                                "p (oh hh ow ww) -> p oh hh ow ww",
                                oh=ohc, hh=psz, ow=ohb, ww=psz)
                            for ci in range(NCH[br]):
                                wy, wx = divmod(ci, psz)
                                dst = xw[cb][:, ci * ntf + chq * tpc:
                                             ci * ntf + (chq + 1) * tpc
                                             ].rearrange("p (a c) -> p a c",
                                                         a=ohc)
                                copy_rr(dst, xtv[:, :, wy, :, wx])
                    for ti in range(ntiles):
                        for (pkf, f0, m, off) in sub[ti]:
                            if pkf != kf:
                                continue
                            for ci in range(NCH[br]):
                                ps = p_vps.tile([128, 128], F32,
                                                name=f"vps{ci % 2}",
                                                tag=f"vps{ci % 2}")
                                for cb in range(2):
                                    lhsT = xw[cb][:, ci * ntf + f0:
                                                  ci * ntf + f0 + m]
                                    nc.tensor.matmul(
                                        ps[off:off + m], lhsT,
                                        wv_bf[cb][:, br * 128:(br + 1) * 128],
                                        start=(cb == 0), stop=(cb == 1),
                                        tile_position=(0, off))
                                dst = v_t[ti][off:off + m,
                                              ci * 128:(ci + 1) * 128]
                                alt[0] ^= 1
                                if alt[0]:
                                    nc.scalar.copy(dst, ps[off:off + m, :])
                                else:
                                    nc.vector.tensor_copy(dst,
                                                          ps[off:off + m, :])

            # --- PV: y^T accumulated over all key tiles; write into att ---
            esA2 = ExitStack()
            if br == 0:
                p_att = esA2.enter_context(tc.tile_pool(name="att0", bufs=1))
            else:
                p_att = p_att1
            att = p_att.tile([128, 98 * 98], F32R, name=f"att{br}",
                             tag=f"att{br}")
            att_sb[br] = att
            attv = att.rearrange("p (h w) -> p h w", h=98)
            nc.scalar.copy(att[:, 0:98], zrow)
            nc.scalar.copy(att[:, 97 * 98:98 * 98], zrow)
            zcol = zrow[:, 0:96].rearrange("p (a c) -> p a c", a=96)
            nc.vector.tensor_copy(attv[:, 1:97, 0:1], zcol)
            nc.vector.tensor_copy(attv[:, 1:97, 97:98], zcol)
            wvw = attv[:, 1:97, 1:97].rearrange(
                "p (oh hh) (ow ww) -> p oh hh ow ww", hh=psz, ww=psz)
            nqh_n = 2 if br == 0 else 1
            nqw = NQ[br] // nqh_n
            ohq = ohb // nqh_n
            with tc.tile_pool(name=f"pvps{br}", bufs=2,
                              space="PSUM") as p_pvps:
                for ci in range(NCH[br]):
                    wy, wx = divmod(ci, psz)
                    for nqh in range(nqh_n):
                        ps = p_pvps.tile([128, nqw], F32, name="pvps",
                                         tag="pvps")
                        for ti in range(ntiles):
                            nc.tensor.matmul(
                                ps, v_t[ti][:, ci * 128:(ci + 1) * 128],
                                pt_t[br][ti][:, nqh * nqw:(nqh + 1) * nqw],
                                start=(ti == 0), stop=(ti == ntiles - 1))
                        dst = wvw[:, nqh * ohq:(nqh + 1) * ohq, wy, :, wx]
                        src = ps.rearrange("p (a c) -> p a c", a=ohq)
                        bias_copy_alt(dst, src, bv_sb[:, br:br + 1])
            if br == 0:
                nc.sync.dma_start(out=att0_dram, in_=att)
                esA2.close()
            esV.close()
            if br == 0:
                esPT0.close()

        # ---------------- phase D: 3x3 conv + LeakyReLU ----------------
        if "D" not in PHASES:
            esAtt1.close(); esPT1.close()
            return nc
        with tc.tile_pool(name="attr", bufs=1) as p_attr, \
             tc.tile_pool(name="wot", bufs=1) as p_wot, \
             tc.tile_pool(name="dout", bufs=3) as p_do, \
             tc.tile_pool(name="dps", bufs=4, space="PSUM") as p_dps:
            att0 = p_attr.tile([128, 98 * 98], F32R, name="attr0", tag="attr0")
            nc.sync.dma_start(out=att0, in_=att0_dram)
            att_in = [att0, att_sb[1]]
            wot_sb = []
            for cb in range(2):
                t = p_wot.tile([128, 9, C], F32R, name=f"wot{cb}",
                               tag=f"wot{cb}")
                nc.sync.dma_start(
                    out=t,
                    in_=wot.ap()[:, cb * 128:(cb + 1) * 128, :].rearrange(
                        "t i o -> i t o"))
                wot_sb.append(t)
            attv2 = [att_in[cb].rearrange("p (h w) -> p h w", h=98)
                     for cb in range(2)]
            for coutb in range(2):
                for rg in range(24):
                    ps = p_dps.tile([128, 384], F32, name="dps", tag="dps")
                    k = 0
                    for cb in range(2):
                        for tap in range(9):
                            dy, dx = divmod(tap, 3)
                            rhs = attv2[cb][:, rg * 4 + dy:rg * 4 + dy + 4,
                                            dx:dx + 96]
                            lhsT = wot_sb[cb][:, tap,
                                              coutb * 128:(coutb + 1) * 128]
                            nc.tensor.matmul(ps, lhsT, rhs,
                                             start=(k == 0), stop=(k == 17))
                            k += 1
                    t1 = p_do.tile([128, 384], F32, name="t1", tag="t1")
                    nc.scalar.activation(out=t1, in_=ps, func=Identity,
                                         bias=bo_sb[:, coutb:coutb + 1],
                                         scale=1.0)
                    t2 = p_do.tile([128, 384], F32, name="t2", tag="t2")
                    nc.vector.scalar_tensor_tensor(
                        out=t2, in0=t1, scalar=0.2, in1=t1,
                        op0=mybir.AluOpType.mult, op1=mybir.AluOpType.max)
                    nc.sync.dma_start(
                        out=out.ap()[coutb * 128:(coutb + 1) * 128,
                                     rg * 384:(rg + 1) * 384],
                        in_=t2)
        esAtt1.close()
        esPT1.close()
    return nc


_CACHED = {}


def _get_nc():
    if "nc" not in _CACHED:
        nc = bacc.Bacc("TRN2", debug=False, target_bir_lowering=False)
        build(nc)
        nc.compile()
        _CACHED["nc"] = nc
    return _CACHED["nc"]


def make_in_maps(x, wq, bq_, wk, bk_, wv, bv_, wo, bo_):
    shared = {
        "wqt": np.ascontiguousarray(wq.T.astype(np.float32)),
        "wkt": np.ascontiguousarray(wk.T.astype(np.float32)),
        "wvt": np.ascontiguousarray(wv.T.astype(np.float32)),
        "wot": np.ascontiguousarray(
            wo.transpose(2, 3, 1, 0).reshape(9, C, C).astype(np.float32)),
        "bq": np.ascontiguousarray(bq_.astype(np.float32)),
        "bk": np.ascontiguousarray(bk_.astype(np.float32)),
        "bv": np.ascontiguousarray(bv_.astype(np.float32)),
        "bo": np.ascontiguousarray(bo_.astype(np.float32)),
    }
    x3 = np.ascontiguousarray(x.reshape(2 * T, C, PIX).astype(np.float32))
    in_maps = []
    for core in range(NCORES):
        v, f = divmod(core, T)
        m = dict(shared)
        m["xv"] = np.ascontiguousarray(x3[v * T:(v + 1) * T])
        m["xf"] = np.ascontiguousarray(x3[v * T + f])
        in_maps.append(m)
    return in_maps


def kernel(**inputs):
    from concourse.bass_utils import run_bass_kernel_spmd

    x = np.asarray(inputs["x"], dtype=np.float32)
    in_maps = make_in_maps(
        x, np.asarray(inputs["wq"]), np.asarray(inputs["bq"]),
        np.asarray(inputs["wk"]), np.asarray(inputs["bk"]),
        np.asarray(inputs["wv"]), np.asarray(inputs["bv"]),
        np.asarray(inputs["wo"]), np.asarray(inputs["bo"]))
    nc = _get_nc()
    res = run_bass_kernel_spmd(nc, in_maps, core_ids=list(range(NCORES)))
    outs = [res.results[c]["out"].reshape(C, H, W) for c in range(NCORES)]
    return np.stack(outs).astype(np.float32)



# revision 4
# speedup vs baseline: 1.3003x; 1.3003x over previous
"""Trainium2 Bass kernel for nn_MultiHeadedAttention_6416681140387.

Two-branch windowed video attention:
  x [8,256,96,96] -> 1x1 conv Q/K/V -> per-branch full attention over
  window-token features (branch0: 4x4 patches, d=2048, 2304 key tokens;
  branch1: 8x8 patches, d=8192, 576 key tokens) -> concat channels
  -> 3x3 conv + LeakyReLU(0.2).

Sharding: 8 cores = (video b in {0,1}) x (frame t in {0..3}). Each core
computes its full output frame [256,96,96]. The host permutes each core's
4-frame video slice so the core's own frame comes first; attention is
key-order invariant so P columns / V rows just follow processing order.

Design (gather-once, window-major convs):
  Per branch pass, per frame: x is gathered once into window-major xw
  (fp16), and Q (frame 0 only) / K / V 1x1-convs all consume xw. K conv
  output is therefore window-major => scores matmuls get contiguous rhs.
  V tiles ([token, d] layout) are spilled to a DRAM scratch and streamed
  back for the PV phase (all-SBUF working set stays under the 192KB cap).
  Whole 16-bit path is fp16 (better mantissa than bf16, same PE rate).
"""

import sys

if "/opt/trn_rl_repo" not in sys.path:
    sys.path.insert(0, "/opt/trn_rl_repo")

import math
from contextlib import ExitStack

import numpy as np

import concourse.bass as bass
import concourse.tile as tile
from concourse import bacc, mybir
from concourse.masks import make_identity

F32 = mybir.dt.float32
FP16 = mybir.dt.float16

T = 4
C = 256
H = W = 96
PIX = H * W
NCORES = 8

PSZ = [4, 8]
OHB = [24, 12]                  # token grid side per branch
NTF = [576, 144]                # tokens per frame
NKP = [2304, 576]               # key tokens per video (no padding)
NCH = [16, 64]                  # d-chunks (psz^2)
NTILE = [18, 5]                 # ceil(NKP/128)
SC = [1.0 / math.sqrt(2048.0), 1.0 / math.sqrt(8192.0)]
NQB = [[(0, 128), (128, 128), (256, 128), (384, 128), (512, 64)],
       [(0, 128), (128, 16)]]

Exp = mybir.ActivationFunctionType.Exp
Identity = mybir.ActivationFunctionType.Identity


def build(nc):
    xv = nc.dram_tensor("xv", [T, C, PIX], F32, kind="ExternalInput")
    wqt = nc.dram_tensor("wqt", [C, C], F32, kind="ExternalInput")
    wkt = nc.dram_tensor("wkt", [C, C], F32, kind="ExternalInput")
    wvt = nc.dram_tensor("wvt", [C, C], F32, kind="ExternalInput")
    wot = nc.dram_tensor("wot", [9, C, C], F32, kind="ExternalInput")
    bq = nc.dram_tensor("bq", [C], F32, kind="ExternalInput")
    bk = nc.dram_tensor("bk", [C], F32, kind="ExternalInput")
    bv = nc.dram_tensor("bv", [C], F32, kind="ExternalInput")
    bo = nc.dram_tensor("bo", [C], F32, kind="ExternalInput")
    out = nc.dram_tensor("out", [C, PIX], F32, kind="ExternalOutput")

    alt = [0]

    def bias_copy_alt(dst, src, bias_ap):
        alt[0] ^= 1
        if alt[0]:
            nc.scalar.activation(out=dst, in_=src, func=Identity,
                                 bias=bias_ap, scale=1.0)
        else:
            nc.vector.tensor_scalar_add(dst, src, bias_ap)

    rr = [0]

    def copy_rr(dst, src):
        rr[0] = (rr[0] + 1) % 3
        if rr[0] == 0:
            nc.vector.tensor_copy(dst, src)
        elif rr[0] == 1:
            nc.scalar.copy(dst, src)
        else:
            nc.gpsimd.tensor_copy(dst, src)

    sv = [0]

    def copy_sv(dst, src):
        # PSUM sources: scalar/vector only (gpsimd can't read PSUM)
        sv[0] ^= 1
        if sv[0]:
            nc.scalar.copy(dst, src)
        else:
            nc.vector.tensor_copy(dst, src)

    with tile.TileContext(nc, pool_alloc_mode="queue") as tc, ExitStack() as top:
        persist = top.enter_context(tc.tile_pool(name="persist", bufs=1))
        dramp = top.enter_context(tc.tile_pool(name="dram", bufs=1, space="DRAM"))

        # fp16 weights (cast from f32 loads)
        wq_sb, wk_sb, wv_sb = [None, None], [None, None], [None, None]
        with tc.tile_pool(name="wload", bufs=2) as p_wl:
            for name, dt_, lst in (("wq", wqt, wq_sb), ("wk", wkt, wk_sb),
                                   ("wv", wvt, wv_sb)):
                for cb in range(2):
                    tf = p_wl.tile([128, C], F32, name="wl", tag="wl")
                    nc.sync.dma_start(out=tf,
                                      in_=dt_.ap()[cb * 128:(cb + 1) * 128, :])
                    t = persist.tile([128, C], FP16, name=f"{name}{cb}",
                                     tag=f"{name}{cb}")
                    nc.vector.tensor_copy(t, tf)
                    lst[cb] = t

        def bias_tile(name, dt_):
            t = persist.tile([128, 2], F32, name=name, tag=name)
            nc.sync.dma_start(
                out=t, in_=bass.AP(tensor=dt_.ap().tensor, offset=0,
                                   ap=[[1, 128], [128, 2]]))
            return t

        bq_sb = bias_tile("bq", bq)
        bk_sb = bias_tile("bk", bk)
        bo_sb = bias_tile("bo", bo)
        bv_sb = bias_tile("bv", bv)
        ident = persist.tile([128, 128], FP16, name="ident", tag="ident")
        make_identity(nc, ident)
        zrow = persist.tile([128, 98], FP16, name="zrow", tag="zrow")
        nc.vector.memset(zrow, 0.0)

        # DRAM scratch: V in [token, d] layout per branch + att0 spill
        vdram = [dramp.tile([NKP[0], NCH[0] * 128], FP16, name="vd0", tag="vd0"),
                 dramp.tile([NKP[1], NCH[1] * 128], FP16, name="vd1", tag="vd1")]
        att0_dram = dramp.tile([128, 98 * 98], FP16, name="att0d", tag="att0d")

        # att1 outlives pass 1 (used by phase D) -> open before the passes
        esAtt1 = ExitStack()
        p_att1 = esAtt1.enter_context(tc.tile_pool(name="att1", bufs=1))
        att_sb = {}

        for br in range(2):
            psz, ohb, ntf, nch = PSZ[br], OHB[br], NTF[br], NCH[br]
            nkp, ntile = NKP[br], NTILE[br]
            nmk = 2 if br == 0 else 1
            mkw = ntf // nmk            # 288 / 144

            es_pt = ExitStack()
            p_pt = es_pt.enter_context(tc.tile_pool(name=f"pt{br}", bufs=1))
            pt_t = [p_pt.tile([128, ntf], FP16, name=f"pt{br}_{i}",
                              tag=f"pt{br}_{i}") for i in range(ntile)]
            es_P = ExitStack()
            p_P = es_P.enter_context(tc.tile_pool(name=f"P{br}", bufs=1))
            p_t = [p_P.tile([128, nkp], FP16, name=f"p{br}_{i}",
                            tag=f"p{br}_{i}") for i in range(len(NQB[br]))]
            p_run = es_P.enter_context(tc.tile_pool(name=f"run{br}", bufs=1))
            run_mx = [p_run.tile([128, 1], F32, name=f"mx{i}", tag=f"mx{i}")
                      for i in range(len(NQB[br]))]
            run_ls = [p_run.tile([128, 1], F32, name=f"ls{i}", tag=f"ls{i}")
                      for i in range(len(NQB[br]))]
            es_qw = ExitStack()
            p_qw = es_qw.enter_context(tc.tile_pool(name=f"qw{br}", bufs=1))
            qw = p_qw.tile([128, nch * ntf], FP16, name=f"qw{br}",
                           tag=f"qw{br}")
            p_stat = es_qw.enter_context(tc.tile_pool(name=f"stat{br}",
                                                      bufs=4))

            # x chunking: 3 / 2 patch-rows per chunk
            ohc = 3 if br == 0 else 2
            nql = ohb // ohc            # 8 / 6 chunks per frame
            csz = ohc * psz * W         # 1152 / 1536 pixels
            vseg = 1 if br == 0 else 2  # V staging column segments
            vsw = nch * 128 // vseg     # 2048 / 4096 cols per segment

            with tc.tile_pool(name=f"xc{br}", bufs=2) as p_xc, \
                 tc.tile_pool(name=f"xw{br}", bufs=2) as p_xw, \
                 tc.tile_pool(name=f"kw{br}", bufs=1) as p_kw, \
                 tc.tile_pool(name=f"vs{br}", bufs=2) as p_vs, \
                 tc.tile_pool(name=f"cps{br}", bufs=2, space="PSUM") as p_cps, \
                 tc.tile_pool(name=f"sps{br}", bufs=2, space="PSUM") as p_sps, \
                 tc.tile_pool(name=f"vps{br}", bufs=2, space="PSUM") as p_vps:
                for pos in range(T):
                    # ---- gather x into window-major xw (fp16) ----
                    xw = [p_xw.tile([128, nch * ntf], FP16, name=f"xw{cb}",
                                    tag=f"xw{cb}") for cb in range(2)]
                    xwv = [xw[cb].rearrange(
                        "p (wy wx oh ow) -> p wy wx oh ow",
                        wy=psz, wx=psz, oh=ohb, ow=ohb) for cb in range(2)]
                    for chq in range(nql):
                        for cb in range(2):
                            xc = p_xc.tile([128, csz], F32, name=f"xc{cb}",
                                           tag=f"xc{cb}")
                            nc.sync.dma_start(
                                out=xc,
                                in_=xv.ap()[pos, cb * 128:(cb + 1) * 128,
                                            chq * csz:(chq + 1) * csz])
                            xcv = xc.rearrange(
                                "p (oh hh ow ww) -> p oh hh ow ww",
                                oh=ohc, hh=psz, ow=ohb, ww=psz)
                            for wy in range(psz):
                                src = xcv[:, :, wy, :, :].rearrange(
                                    "p a b c -> p c a b")
                                dst = xwv[cb][:, wy, :,
                                              chq * ohc:(chq + 1) * ohc, :]
                                copy_rr(dst, src)

                    # ---- Q conv (own frame only) and K conv from xw ----
                    def conv_from_xw(w_sb, b_sb, dst_tile):
                        for k in range(nch * ntf // 512):
                            ps = p_cps.tile([128, 512], F32, name="cps",
                                            tag="cps")
                            for cb in range(2):
                                nc.tensor.matmul(
                                    ps,
                                    w_sb[cb][:, br * 128:(br + 1) * 128],
                                    xw[cb][:, k * 512:(k + 1) * 512],
                                    start=(cb == 0), stop=(cb == 1))
                            bias_copy_alt(dst_tile[:, k * 512:(k + 1) * 512],
                                          ps, b_sb[:, br:br + 1])

                    if pos == 0:
                        conv_from_xw(wq_sb, bq_sb, qw)
                    kw = p_kw.tile([128, nch * ntf], FP16, name="kw", tag="kw")
                    conv_from_xw(wk_sb, bk_sb, kw)

                    # ---- scores + online softmax ----
                    for nqi, (q0, nqsz) in enumerate(NQB[br]):
                        pss = [p_sps.tile([128, mkw], F32, name=f"s{mkh}",
                                          tag=f"s{mkh}")
                               for mkh in range(nmk)]
                        for ci in range(nch):
                            for mkh in range(nmk):
                                nc.tensor.matmul(
                                    pss[mkh][:nqsz],
                                    qw[:, ci * ntf + q0:ci * ntf + q0 + nqsz],
                                    kw[:, ci * ntf + mkh * mkw:
                                       ci * ntf + (mkh + 1) * mkw],
                                    start=(ci == 0), stop=(ci == nch - 1))
                        for mkh in range(nmk):
                            ps = pss[mkh]
                            o = pos * ntf + mkh * mkw
                            pt = p_t[nqi]
                            mx, ls = run_mx[nqi], run_ls[nqi]
                            bm = p_stat.tile([128, 1], F32, name="bm",
                                             tag="bm")
                            nc.vector.reduce_max(out=bm[:nqsz],
                                                 in_=ps[:nqsz, :],
                                                 axis=mybir.AxisListType.X)
                            if pos == 0 and mkh == 0:
                                nc.vector.tensor_copy(mx[:nqsz], bm[:nqsz])
                                nmx = p_stat.tile([128, 1], F32, name="nmx",
                                                  tag="nmx")
                                nc.vector.tensor_scalar_mul(
                                    nmx[:nqsz], mx[:nqsz], -SC[br])
                                nc.scalar.activation(
                                    out=pt[:nqsz, o:o + mkw],
                                    in_=ps[:nqsz, :], func=Exp,
                                    bias=nmx[:nqsz], scale=SC[br],
                                    accum_out=ls[:nqsz])
                            else:
                                nmax = p_stat.tile([128, 1], F32,
                                                   name="nmax", tag="nmax")
                                nc.vector.tensor_max(nmax[:nqsz], mx[:nqsz],
                                                     bm[:nqsz])
                                nmx = p_stat.tile([128, 1], F32, name="nmx",
                                                  tag="nmx")
                                nc.vector.tensor_scalar_mul(
                                    nmx[:nqsz], nmax[:nqsz], -SC[br])
                                delta = p_stat.tile([128, 1], F32,
                                                    name="delta", tag="delta")
                                nc.scalar.activation(
                                    out=delta[:nqsz], in_=mx[:nqsz],
                                    func=Exp, bias=nmx[:nqsz], scale=SC[br])
                                nc.vector.tensor_scalar_mul(
                                    pt[:nqsz, 0:o], pt[:nqsz, 0:o],
                                    delta[:nqsz])
                                pl = p_stat.tile([128, 1], F32, name="pl",
                                                 tag="pl")
                                nc.scalar.activation(
                                    out=pt[:nqsz, o:o + mkw],
                                    in_=ps[:nqsz, :], func=Exp,
                                    bias=nmx[:nqsz], scale=SC[br],
                                    accum_out=pl[:nqsz])
                                nc.vector.scalar_tensor_tensor(
                                    out=ls[:nqsz], in0=ls[:nqsz],
                                    scalar=delta[:nqsz], in1=pl[:nqsz],
                                    op0=mybir.AluOpType.mult,
                                    op1=mybir.AluOpType.add)
                                nc.vector.tensor_copy(mx[:nqsz], nmax[:nqsz])

                    # ---- V conv from xw -> DRAM scratch ----
                    for gi, (g0, gm) in enumerate(NQB[br]):
                        for seg in range(vseg):
                            vstage = p_vs.tile([128, vsw], FP16,
                                               name="vs", tag="vs")
                            nseg = nch // vseg
                            for cis in range(nseg):
                                ci = seg * nseg + cis
                                ps = p_vps.tile([128, 128], F32, name="vps",
                                                tag="vps")
                                for cb in range(2):
                                    nc.tensor.matmul(
                                        ps[:gm],
                                        xw[cb][:, ci * ntf + g0:
                                               ci * ntf + g0 + gm],
                                        wv_sb[cb][:, br * 128:(br + 1) * 128],
                                        start=(cb == 0), stop=(cb == 1))
                                copy_sv(vstage[:gm,
                                               cis * 128:(cis + 1) * 128],
                                        ps[:gm, :])
                            row0 = pos * ntf + g0
                            nc.sync.dma_start(
                                out=vdram[br][row0:row0 + gm,
                                              seg * vsw:(seg + 1) * vsw],
                                in_=vstage[:gm, :])

                # ---- finalize softmax: P /= ls ----
                for nqi, (q0, nqsz) in enumerate(NQB[br]):
                    rs = p_stat.tile([128, 1], F32, name="rs", tag="rs")
                    nc.vector.reciprocal(rs[:nqsz], run_ls[nqi][:nqsz])
                    nc.vector.tensor_scalar_mul(
                        p_t[nqi][:nqsz, :], p_t[nqi][:nqsz, :], rs[:nqsz])
            es_qw.close()

            # ---- P^T transposes ----
            with tc.tile_pool(name=f"tp{br}", bufs=2, space="PSUM") as p_tp:
                for ti in range(ntile):
                    t0 = ti * 128
                    m = min(128, nkp - t0)
                    for nqi, (q0, nqsz) in enumerate(NQB[br]):
                        tp = p_tp.tile([128, 128], FP16, name="tp", tag="tp")
                        nc.tensor.transpose(
                            tp[:m, :nqsz], p_t[nqi][:nqsz, t0:t0 + m],
                            ident[:nqsz, :nqsz])
                        copy_sv(pt_t[ti][:m, q0:q0 + nqsz], tp[:m, :nqsz])
            es_P.close()

            # ---- PV: y^T accumulated over key tiles -> att ----
            es_att0 = ExitStack()
            if br == 0:
                p_att = es_att0.enter_context(
                    tc.tile_pool(name="att0", bufs=1))
            else:
                p_att = p_att1
            att = p_att.tile([128, 98 * 98], FP16, name=f"att{br}",
                             tag=f"att{br}")
            att_sb[br] = att
            attv = att.rearrange("p (h w) -> p h w", h=98)
            nc.scalar.copy(att[:, 0:98], zrow)
            nc.scalar.copy(att[:, 97 * 98:98 * 98], zrow)
            zcol = zrow[:, 0:96].rearrange("p (a c) -> p a c", a=96)
            nc.vector.tensor_copy(attv[:, 1:97, 0:1], zcol)
            nc.vector.tensor_copy(attv[:, 1:97, 97:98], zcol)
            wvw = attv[:, 1:97, 1:97].rearrange(
                "p (oh hh) (ow ww) -> p oh hh ow ww", hh=psz, ww=psz)

            es_vt = ExitStack()
            p_vt = es_vt.enter_context(tc.tile_pool(name=f"vt{br}", bufs=1))
            vt = []
            for ti in range(ntile):
                t0 = ti * 128
                m = min(128, nkp - t0)
                t = p_vt.tile([128, nch * 128], FP16, name=f"vt{ti}",
                              tag=f"vt{ti}")
                nc.sync.dma_start(out=t[:m, :], in_=vdram[br][t0:t0 + m, :])
                vt.append(t)

            nqh_n = 2 if br == 0 else 1
            nqw = ntf // nqh_n
            ohq = ohb // nqh_n
            with tc.tile_pool(name=f"pv{br}", bufs=2, space="PSUM") as p_pv:
                for ci in range(nch):
                    wy, wx = divmod(ci, psz)
                    for nqh in range(nqh_n):
                        ps = p_pv.tile([128, nqw], F32, name="pv", tag="pv")
                        for ti in range(ntile):
                            m = min(128, nkp - ti * 128)
                            nc.tensor.matmul(
                                ps, vt[ti][:m, ci * 128:(ci + 1) * 128],
                                pt_t[ti][:m, nqh * nqw:(nqh + 1) * nqw],
                                start=(ti == 0), stop=(ti == ntile - 1))
                        dst = wvw[:, nqh * ohq:(nqh + 1) * ohq, wy, :, wx]
                        src = ps.rearrange("p (a c) -> p a c", a=ohq)
                        bias_copy_alt(dst, src, bv_sb[:, br:br + 1])
            es_vt.close()
            if br == 0:
                nc.sync.dma_start(out=att0_dram, in_=att)
                es_att0.close()
            es_pt.close()

        # ---------------- phase D: 3x3 conv + LeakyReLU ----------------
        with tc.tile_pool(name="attr", bufs=1) as p_attr, \
             tc.tile_pool(name="wot", bufs=1) as p_wot, \
             tc.tile_pool(name="wotl", bufs=2) as p_wotl, \
             tc.tile_pool(name="dout", bufs=3) as p_do, \
             tc.tile_pool(name="dps", bufs=4, space="PSUM") as p_dps:
            att0 = p_attr.tile([128, 98 * 98], FP16, name="attr0",
                               tag="attr0")
            nc.sync.dma_start(out=att0, in_=att0_dram)
            att_in = [att0, att_sb[1]]
            wot_sb = []
            for cb in range(2):
                tf = p_wotl.tile([128, 9 * C], F32, name="wotl", tag="wotl")
                nc.sync.dma_start(
                    out=tf.rearrange("i (t o) -> i t o", t=9),
                    in_=wot.ap()[:, cb * 128:(cb + 1) * 128, :].rearrange(
                        "t i o -> i t o"))
                t = p_wot.tile([128, 9, C], FP16, name=f"wot{cb}",
                               tag=f"wot{cb}")
                nc.vector.tensor_copy(t, tf.rearrange("i (t o) -> i t o",
                                                      t=9))
                wot_sb.append(t)
            attv2 = [att_in[cb].rearrange("p (h w) -> p h w", h=98)
                     for cb in range(2)]
            for coutb in range(2):
                for rg in range(24):
                    ps = p_dps.tile([128, 384], F32, name="dps", tag="dps")
                    k = 0
                    for cb in range(2):
                        for tap in range(9):
                            dy, dx = divmod(tap, 3)
                            rhs = attv2[cb][:, rg * 4 + dy:rg * 4 + dy + 4,
                                            dx:dx + 96]
                            lhsT = wot_sb[cb][:, tap,
                                              coutb * 128:(coutb + 1) * 128]
                            nc.tensor.matmul(ps, lhsT, rhs,
                                             start=(k == 0), stop=(k == 17))
                            k += 1
                    t1 = p_do.tile([128, 384], F32, name="t1", tag="t1")
                    nc.scalar.activation(out=t1, in_=ps, func=Identity,
                                         bias=bo_sb[:, coutb:coutb + 1],
                                         scale=1.0)
                    t2 = p_do.tile([128, 384], F32, name="t2", tag="t2")
                    nc.vector.scalar_tensor_tensor(
                        out=t2, in0=t1, scalar=0.2, in1=t1,
                        op0=mybir.AluOpType.mult, op1=mybir.AluOpType.max)
                    nc.sync.dma_start(
                        out=out.ap()[coutb * 128:(coutb + 1) * 128,
                                     rg * 384:(rg + 1) * 384],
                        in_=t2)
        esAtt1.close()
    return nc


_CACHED = {}


def _get_nc():
    if "nc" not in _CACHED:
        nc = bacc.Bacc("TRN2", debug=False, target_bir_lowering=False)
        build(nc)
        nc.compile()
        _CACHED["nc"] = nc
    return _CACHED["nc"]


def make_in_maps(x, wq, bq_, wk, bk_, wv, bv_, wo, bo_):
    shared = {
        "wqt": np.ascontiguousarray(wq.T.astype(np.float32)),
        "wkt": np.ascontiguousarray(wk.T.astype(np.float32)),
        "wvt": np.ascontiguousarray(wv.T.astype(np.float32)),
        "wot": np.ascontiguousarray(
            wo.transpose(2, 3, 1, 0).reshape(9, C, C).astype(np.float32)),
        "bq": np.ascontiguousarray(bq_.astype(np.float32)),
        "bk": np.ascontiguousarray(bk_.astype(np.float32)),
        "bv": np.ascontiguousarray(bv_.astype(np.float32)),
        "bo": np.ascontiguousarray(bo_.astype(np.float32)),
    }
    x3 = np.ascontiguousarray(x.reshape(2 * T, C, PIX).astype(np.float32))
    in_maps = []
    for core in range(NCORES):
        v, f = divmod(core, T)
        perm = [f] + [g for g in range(T) if g != f]
        m = dict(shared)
        m["xv"] = np.ascontiguousarray(x3[v * T:(v + 1) * T][perm])
        in_maps.append(m)
    return in_maps


def kernel(**inputs):
    from concourse.bass_utils import run_bass_kernel_spmd

    x = np.asarray(inputs["x"], dtype=np.float32)
    in_maps = make_in_maps(
        x, np.asarray(inputs["wq"]), np.asarray(inputs["bq"]),
        np.asarray(inputs["wk"]), np.asarray(inputs["bk"]),
        np.asarray(inputs["wv"]), np.asarray(inputs["bv"]),
        np.asarray(inputs["wo"]), np.asarray(inputs["bo"]))
    nc = _get_nc()
    res = run_bass_kernel_spmd(nc, in_maps, core_ids=list(range(NCORES)))
    outs = [res.results[c]["out"].reshape(C, H, W) for c in range(NCORES)]
    return np.stack(outs).astype(np.float32)
